# revision 27
# baseline (speedup 1.0000x reference)
"""Multi-head causal attention (B=4, N=2048, C=1024, H=16) on 8 trn2 NeuronCores.

Sharding: core c -> batch b = c//2, head-group g = c%2 (8 heads each).
Each core computes qkv projection for its heads, causal attention, and a
partial output projection over its 512 attention channels; a pair-wise
ReduceScatter(add) completes the projection, each core emitting its half of
the tokens for its batch.  Host assembles the 8 results.

v3: single software-pipelined stream.  The ScalarE exp (~157us) gates the
attention inner loop, so QKV(g+1)/proj(g-1) matmul groups are statically
interleaved into the attention kt-loops: PE fills its exp-wait gaps with
projection work and ScalarE never starves.  Causal masking moved off the
DVE into a PE accumulate-matmul (identity @ mask) into the score psum, so
exp feeds attnV directly.  Both heads' attention outputs land in one
[128,1024] psum tile: one reciprocal per pair straight from psum, one
gpsimd row-broadcast.  PSUM: scores 2x[128,1024] + ao 1x[128,1024] +
rotation 2x[128,512] = 8 banks.
"""

import os
import sys

for _p in ("/opt/trn_rl_repo",):
    if _p not in sys.path:
        sys.path.insert(0, _p)

import numpy as np

B = 4
N = 2048
C = 1024
H = 16
DK = 64
N_CORES = 8
HL = 8  # local heads per core
CL = HL * DK  # 512 local channels
PAIRS = HL // 2  # local head pairs
NT = N // 128  # 16 token tiles of 128
NQ = N // 512  # 4 query chunks of 512 (= pipeline groups)
KC = C // 128  # 8 embed contraction chunks

_nc_cache = None


def _build():
    import concourse.bass as bass
    import concourse.mybir as mybir
    import concourse.tile as tile
    from concourse import bacc
    from contextlib import ExitStack

    f32 = mybir.dt.float32
    bf16 = mybir.dt.bfloat16

    nc = bacc.Bacc("TRN2", target_bir_lowering=False, num_devices=N_CORES)

    xt_d = nc.dram_tensor("xt", [128, KC, N], bf16, kind="ExternalInput")
    w_q = nc.dram_tensor("w_q", [128, PAIRS, KC, 128], bf16, kind="ExternalInput")
    w_k = nc.dram_tensor("w_k", [128, PAIRS, KC, 128], bf16, kind="ExternalInput")
    w_v = nc.dram_tensor("w_v", [128, KC, CL], bf16, kind="ExternalInput")
    w_o = nc.dram_tensor("w_o", [128, PAIRS, C], bf16, kind="ExternalInput")
    b_q = nc.dram_tensor("b_q", [128, PAIRS], f32, kind="ExternalInput")
    b_o2 = nc.dram_tensor("b_o2", [1, C], f32, kind="ExternalInput")
    idm_d = nc.dram_tensor("idm", [128, 384], bf16, kind="ExternalInput")
    out = nc.dram_tensor("out", [768 + 512, C], bf16, kind="ExternalOutput")

    EXP = mybir.ActivationFunctionType.Exp

    with tile.TileContext(nc, pool_alloc_mode="queue") as tc, ExitStack() as st:
        # ---------- permanent pools ----------
        const = st.enter_context(tc.tile_pool(name="const", bufs=1))
        idm_sb = const.tile([128, 384], bf16)  # [0:128]=I, [128:256]=mask, [256:384]=tri
        bq_sb = const.tile([128, PAIRS], f32)
        bo_bc = const.tile([128, C], f32)
        dmy = const.tile([1, 8], f32)

        ident = idm_sb[:, 0:128]
        maskm = idm_sb[:, 128:256]
        tri_sb = idm_sb[:, 256:384]
        tri2 = bass.AP(
            tensor=idm_sb.tensor,
            offset=idm_sb.offset + 256,
            ap=[list(idm_sb.ap[0]), [0, 2], [1, 128]],
        )

        # resident weights (bf16), one contiguous tile each -> one DMA each
        w_pool = st.enter_context(tc.tile_pool(name="w", bufs=1))
        wq_all = w_pool.tile([128, PAIRS, KC, 128], bf16, tag="wq", name="wq")
        wk_all = w_pool.tile([128, PAIRS, KC, 128], bf16, tag="wk", name="wk")
        wv_all = w_pool.tile([128, KC, CL], bf16, tag="wv", name="wv")
        wo_all = w_pool.tile([128, PAIRS, C], bf16, tag="wo", name="wo")

        # persistent activations
        act = st.enter_context(tc.tile_pool(name="act", bufs=1))
        kT = [act.tile([128, N], bf16, tag=f"kT{p}", name=f"kT{p}") for p in range(PAIRS)]
        vt = act.tile([128, NT, HL, DK + 1], bf16, tag="vt", name="vt")
        # double-buffered by g-parity (written for g+1 while g is read)
        qT = [
            [act.tile([128, 512], bf16, tag=f"qT{p}_{s}", name=f"qT{p}_{s}") for p in range(PAIRS)]
            for s in range(2)
        ]
        aoT = [
            [act.tile([128, 512], bf16, tag=f"aoT{p}_{s}", name=f"aoT{p}_{s}") for p in range(PAIRS)]
            for s in range(2)
        ]

        xt_pool = st.enter_context(tc.tile_pool(name="xt", bufs=1))
        pt_pool = st.enter_context(tc.tile_pool(name="pt", bufs=8))
        nrm_pool = st.enter_context(tc.tile_pool(name="nrm", bufs=2))
        ob_pool = st.enter_context(tc.tile_pool(name="ob", bufs=3))

        ps = st.enter_context(tc.tile_pool(name="ps", bufs=1, space="PSUM"))
        dram = st.enter_context(tc.tile_pool(name="dram", bufs=1, space="DRAM"))
        rs_in = dram.tile([N, C], bf16, name="rs_in")
        rs_out = dram.tile([N // 2, C], bf16, name="rs_out")

        # ---------- startup DMAs (wv first: V(0) is the first compute) ----------
        xg = {}

        def load_xt(g):
            t = xt_pool.tile([128, KC, 512], bf16, tag="xt", bufs=2, name=f"xt{g}")
            nc.sync.dma_start(out=t, in_=xt_d[:, :, g * 512 : (g + 1) * 512])
            xg[g] = t

        load_xt(0)
        nc.sync.dma_start(out=idm_sb, in_=idm_d[:, :])
        nc.scalar.dma_start(out=wv_all, in_=w_v[:, :, :])
        nc.scalar.dma_start(out=wq_all, in_=w_q[:, :, :, :])
        nc.gpsimd.dma_start(out=wk_all, in_=w_k[:, :, :, :])
        nc.sync.dma_start(out=bq_sb, in_=b_q[:, :])
        load_xt(1)
        nc.gpsimd.dma_start(out=wo_all, in_=w_o[:, :, :])
        nc.gpsimd.dma_start(out=bo_bc, in_=b_o2[0:1, :].partition_broadcast(128))

        nc.vector.memset(vt[:, :, :, DK : DK + 1], 1.0)
        # preload the exp table set while startup DMAs land
        nc.vector.memset(dmy, 0.0)
        nc.scalar.activation(dmy, dmy, EXP)

        # warm the HAM clock gate while the first DMAs land
        for w in range(16):
            warm = ps.tile([128, 512], f32, tag="rot", bufs=2, name=f"warm{w}")
            nc.tensor.matmul(
                warm[:, 0:128], ident, ident,
                start=True, stop=True, skip_group_check=True,
            )

        # ---------- filler group emitters (QKV / proj on the rotation banks) ----
        def v_group(g, i):
            xs = xg[g]
            mt = 4 * g + i
            pv = ps.tile([128, 512], f32, tag="rot", bufs=2, name=f"pv{mt}")
            mms = []
            for kc in range(KC):
                mms.append(lambda kc=kc, pv=pv, xs=xs, i=i: nc.tensor.matmul(
                    pv[:, :],
                    xs[:, kc, i * 128 : (i + 1) * 128],
                    wv_all[:, kc, :],
                    start=(kc == 0), stop=(kc == KC - 1),
                ))
            def drain(pv=pv, mt=mt):
                nc.vector.tensor_copy(
                    vt[:, mt, :, 0:DK], pv.rearrange("p (h d) -> p h d", h=HL)
                )
            mms.append(drain)
            return mms

        def q_group(g, p):
            xs = xg[g]
            pq = ps.tile([128, 512], f32, tag="rot", bufs=2, name=f"pq{g}_{p}")
            mms = []
            for kc in range(KC):
                mms.append(lambda kc=kc, pq=pq, xs=xs, p=p: nc.tensor.matmul(
                    pq[:, :], wq_all[:, p, kc, :], xs[:, kc, :],
                    start=(kc == 0), stop=(kc == KC - 1),
                ))
            def drain(pq=pq, g=g, p=p):
                nc.vector.tensor_scalar(
                    out=qT[g % 2][p][:, :], in0=pq[:, :],
                    scalar1=bq_sb[:, p : p + 1], scalar2=None,
                    op0=mybir.AluOpType.add,
                )
            mms.append(drain)
            return mms

        def k_group(g, p):
            xs = xg[g]
            pk = ps.tile([128, 512], f32, tag="rot", bufs=2, name=f"pk{g}_{p}")
            mms = []
            for kc in range(KC):
                mms.append(lambda kc=kc, pk=pk, xs=xs, p=p: nc.tensor.matmul(
                    pk[:, :], wk_all[:, p, kc, :], xs[:, kc, :],
                    start=(kc == 0), stop=(kc == KC - 1),
                ))
            def drain(pk=pk, g=g, p=p):
                nc.vector.tensor_copy(kT[p][:, g * 512 : (g + 1) * 512], pk[:, :])
            mms.append(drain)
            return mms

        def proj_group(g, grp):
            # grp in 0..7 -> (i, nn):  token tile i of chunk g, col half nn
            i, nn = divmod(grp, 2)
            mt = 4 * g + i
            pj = ps.tile([128, 512], f32, tag="rot", bufs=2, name=f"pj{mt}_{nn}")
            mms = []
            for cc in range(PAIRS):
                mms.append(lambda cc=cc, pj=pj, g=g, i=i, nn=nn: nc.tensor.matmul(
                    pj[:, :],
                    aoT[g % 2][cc][:, i * 128 : (i + 1) * 128],
                    wo_all[:, cc, nn * 512 : (nn + 1) * 512],
                    start=(cc == 0), stop=(cc == PAIRS - 1),
                ))
            def drain(pj=pj, g=g, mt=mt, nn=nn):
                ob = ob_pool.tile([128, 512], bf16, name="ob")
                nc.vector.tensor_tensor(
                    ob[:, :], pj[:, :], bo_bc[:, nn * 512 : (nn + 1) * 512],
                    mybir.AluOpType.add,
                )
                if g < NQ - 1:
                    nc.sync.dma_start(
                        out=rs_in[mt * 128 : (mt + 1) * 128, nn * 512 : (nn + 1) * 512],
                        in_=ob[:, :],
                    )
                else:
                    nc.sync.dma_start(
                        out=out[
                            768 + (mt - 12) * 128 : 768 + (mt - 11) * 128,
                            nn * 512 : (nn + 1) * 512,
                        ],
                        in_=ob[:, :],
                    )
            mms.append(drain)
            return mms

        def rs_action(g):
            g0 = g * 512
            def run():
                nc.gpsimd.collective_compute(
                    "ReduceScatter",
                    mybir.AluOpType.add,
                    replica_groups=[[0, 1], [2, 3], [4, 5], [6, 7]],
                    ins=[rs_in[g0 : g0 + 512, :].opt()],
                    outs=[rs_out[g0 // 2 : g0 // 2 + 256, :].opt()],
                )
                nc.sync.dma_start(
                    out=out[g0 // 2 : g0 // 2 + 256, :],
                    in_=rs_out[g0 // 2 : g0 // 2 + 256, :],
                )
            return [run]

        # ---------- static filler schedule per (g, p) ----------
        def filler_for(g, p):
            f = []
            if g == 0:
                # startup handles V(0), QK(0,p0/p1); spread the rest
                if p == 0:
                    f += q_group(0, 2) + k_group(0, 2)
                    f += v_group(1, 2) + v_group(1, 3)
                elif p == 1:
                    f += q_group(0, 3) + k_group(0, 3)
                elif p == 2:
                    f += q_group(1, 0) + k_group(1, 0)
                else:
                    f += q_group(1, 1) + k_group(1, 1)
                    f += v_group(1, 0) + v_group(1, 1)
                return f
            if g == 3:
                # no QKV(4): spread QK(3,*) and proj(2) as late as legal, and
                # start RS(2) one pair early so its DMA drains under p3+proj(3)
                if p == 0:
                    f += q_group(3, 1) + k_group(3, 1)
                    f += q_group(3, 2) + k_group(3, 2)
                    f += proj_group(2, 0) + proj_group(2, 1)
                elif p == 1:
                    f += q_group(3, 3) + k_group(3, 3)
                    f += proj_group(2, 2) + proj_group(2, 3) + proj_group(2, 4)
                elif p == 2:
                    f += proj_group(2, 5) + proj_group(2, 6) + proj_group(2, 7)
                    f += rs_action(2)
                return f
            # 1 <= g <= 2
            if p == 0:
                f += q_group(g, 2) + k_group(g, 2)
                f += v_group(g + 1, 2) + v_group(g + 1, 3)
                f += proj_group(g - 1, 0) + proj_group(g - 1, 1)
            elif p == 1:
                f += q_group(g, 3) + k_group(g, 3)
                f += proj_group(g - 1, 2) + proj_group(g - 1, 3) + proj_group(g - 1, 4)
            elif p == 2:
                f += q_group(g + 1, 0) + k_group(g + 1, 0)
                f += proj_group(g - 1, 5) + proj_group(g - 1, 6) + proj_group(g - 1, 7)
            else:
                f += rs_action(g - 1)
                if g < 2:
                    f += q_group(g + 1, 1) + k_group(g + 1, 1)
                f += v_group(g + 1, 0) + v_group(g + 1, 1)
            return f

        # ---------- startup compute: V(0), QK(0, p0/p1) ----------
        for i in range(4):
            for step in v_group(0, i):
                step()
        for step in q_group(0, 0) + k_group(0, 0) + q_group(0, 1) + k_group(0, 1):
            step()

        # ---------- main pipelined attention loop ----------
        for g in range(NQ):
            g0 = g * 512
            for p in range(PAIRS):
                # prefetch late enough that the DMA queue isn't head-of-line
                # blocked on the xt slot's WAR (last reader: QK(g,3) at p=1)
                if p == 2 and g + 2 < NQ:
                    load_xt(g + 2)
                n_kt = 4 * g + 4
                filler = filler_for(g, p)
                fpos = [0]

                def pump(k):
                    for _ in range(k):
                        if fpos[0] < len(filler):
                            filler[fpos[0]]()
                            fpos[0] += 1

                ao = ps.tile([128, 1024], f32, tag="ao", bufs=1, name=f"ao{g}_{p}")
                pts = {}

                def scores(kt):
                    off = 128 * (kt - 4 * g) if kt >= 4 * g else 0
                    s_t = ps.tile([128, 1024], f32, tag="s", bufs=2, name="st")
                    for h in range(2):
                        rows = slice(64 * h, 64 * h + 64)
                        nc.tensor.matmul(
                            s_t[:, 512 * h + off : 512 * h + 512],
                            kT[p][rows, kt * 128 : (kt + 1) * 128],
                            qT[g % 2][p][rows, off:512],
                            start=True, stop=True, tile_position=(64 * h, 0),
                            skip_group_check=True,
                        )
                    pt = pt_pool.tile([128, 1024], bf16, name="pt")
                    if off:
                        s4 = bass.AP(
                            tensor=s_t.tensor,
                            offset=s_t.offset + off,
                            ap=[list(s_t.ap[0]), [512, 2], [1, 512 - off]],
                        )
                        p4 = bass.AP(
                            tensor=pt.tensor,
                            offset=pt.offset + off,
                            ap=[list(pt.ap[0]), [512, 2], [1, 512 - off]],
                        )
                        nc.scalar.activation(p4, s4, EXP, scale=0.125)
                    else:
                        nc.scalar.activation(pt[:, 0:1024], s_t[:, 0:1024], EXP, scale=0.125)
                    if kt >= 4 * g:  # triangular boundary blocks, both heads
                        blk = bass.AP(
                            tensor=pt.tensor,
                            offset=pt.offset + off,
                            ap=[list(pt.ap[0]), [512, 2], [1, 128]],
                        )
                        nc.vector.tensor_tensor(blk, blk, tri2, mybir.AluOpType.mult)
                    pts[kt] = pt

                def attn_v(kt):
                    off = 128 * (kt - 4 * g) if kt >= 4 * g else 0
                    pt = pts.pop(kt)
                    for h in range(2):
                        nc.tensor.matmul(
                            ao[0:65, 512 * h + off : 512 * h + 512],
                            vt[:, kt, 2 * p + h, :],
                            pt[:, 512 * h + off : 512 * h + 512],
                            start=(kt == 0), stop=(kt == n_kt - 1),
                            skip_group_check=True,
                        )

                for kt in range(n_kt):
                    scores(kt)
                    if kt >= 2:
                        attn_v(kt - 2)
                    pump(2 if kt % 2 == 0 else 1)
                attn_v(n_kt - 2)
                attn_v(n_kt - 1)
                pump(len(filler))  # flush this pair's filler

                # softmax normalize: aoT = ao[0:64] * (1/rowsum); rowsum = ao[64]
                rs_row = nrm_pool.tile([1, 1024], f32, tag="rsr", bufs=2, name="rsr")
                nc.vector.tensor_copy(rs_row[0:1, :], ao[64:65, :])
                rcp = nrm_pool.tile([1, 1024], f32, tag="rcp", bufs=2, name="rcp")
                nc.vector.reciprocal_approx_fast(rcp[:, :], rs_row[0:1, :])
                rcpb = nrm_pool.tile([64, 1024], f32, tag="rcpb", bufs=2, name="rcpb")
                nc.gpsimd.partition_broadcast(rcpb[:, :], rcp[0:1, :], channels=64)
                for h in range(2):
                    nc.vector.tensor_tensor(
                        aoT[g % 2][p][64 * h : 64 * h + 64, :],
                        ao[0:64, 512 * h : 512 * h + 512],
                        rcpb[:, 512 * h : 512 * h + 512],
                        mybir.AluOpType.mult,
                    )

        # ---------- tail: proj(3) ----------
        for grp in range(8):
            for step in proj_group(3, grp):
                step()

    nc.compile()
    return nc


def _get_nc():
    global _nc_cache
    if _nc_cache is None:
        _nc_cache = _build()
    return _nc_cache


def kernel(x, W_qkv, b_qkv, W_o, b_o):
    import ml_dtypes
    from concourse.bass_utils import run_bass_kernel_spmd

    bf = ml_dtypes.bfloat16
    x = np.asarray(x, dtype=np.float32)
    W_qkv = np.asarray(W_qkv, dtype=np.float32)
    b_qkv = np.asarray(b_qkv, dtype=np.float32)
    W_o = np.asarray(W_o, dtype=np.float32)
    b_o = np.asarray(b_o, dtype=np.float32)

    # idm: [128, 0:128] identity; [128, 128:256] causal mask: -240 where the
    # [ktok_row, qtok_col] block entry violates j >= i (q < k).
    ident = np.eye(128, dtype=np.float32)
    m = np.where(
        np.arange(128)[None, :] >= np.arange(128)[:, None], 0.0, -240.0
    ).astype(np.float32)
    tri = np.triu(np.ones((128, 128), dtype=np.float32))
    idm = np.concatenate([ident, m, tri], axis=1)

    in_maps = []
    for c in range(N_CORES):
        b, g = divmod(c, 2)
        cs = slice(CL * g, CL * (g + 1))
        W_q_c = W_qkv[:, 0:C][:, cs]
        W_k_c = W_qkv[:, C : 2 * C][:, cs]
        W_v_c = W_qkv[:, 2 * C : 3 * C][:, cs]
        b_v_c = b_qkv[2 * C : 3 * C][cs]
        W_o_c = W_o[cs, :]
        # V-bias folds into the output bias: softmax rows sum to 1, so
        # P @ (1 b_v^T) = 1 b_v^T, and (O + 1 b_v^T) W_o = O W_o + 1 (b_v^T W_o).
        bo2 = 0.5 * b_o + b_v_c @ W_o_c
        in_maps.append(
            {
                "xt": np.ascontiguousarray(
                    x[b].T.reshape(KC, 128, N).transpose(1, 0, 2)
                ).astype(bf),
                "w_q": np.ascontiguousarray(
                    W_q_c.reshape(KC, 128, PAIRS, 128).transpose(1, 2, 0, 3)
                ).astype(bf),
                "w_k": np.ascontiguousarray(
                    W_k_c.reshape(KC, 128, PAIRS, 128).transpose(1, 2, 0, 3)
                ).astype(bf),
                "w_v": np.ascontiguousarray(
                    W_v_c.reshape(KC, 128, CL).transpose(1, 0, 2)
                ).astype(bf),
                "w_o": np.ascontiguousarray(
                    W_o_c.reshape(PAIRS, 128, C).transpose(1, 0, 2)
                ).astype(bf),
                "b_q": np.ascontiguousarray(
                    b_qkv[0:C][cs].reshape(PAIRS, 128).T
                ).astype(np.float32),
                "b_o2": np.ascontiguousarray(bo2[None, :]).astype(np.float32),
                "idm": np.ascontiguousarray(idm).astype(bf),
            }
        )

    nc = _get_nc()
    trace = bool(int(os.environ.get("BASS_KERNEL_TRACE", "0")))
    tmpdir = os.environ.get("BASS_KERNEL_TRACE_DIR") or None
    res = run_bass_kernel_spmd(
        nc, in_maps, list(range(N_CORES)), trace=trace, tmpdir=tmpdir
    )
    kernel.last_result = res

    full = np.empty((B, N, C), dtype=np.float32)
    chunks = [(0, 512), (512, 512), (1024, 512)]
    outs = [np.asarray(res.results[c]["out"], dtype=np.float32) for c in range(N_CORES)]
    for c in range(N_CORES):
        b, rank = divmod(c, 2)
        o = outs[c]
        out_r = 0
        for t0, tn in chunks:
            h = tn // 2
            full[b, t0 + rank * h : t0 + (rank + 1) * h, :] = o[out_r : out_r + h, :]
            out_r += h
    for b in range(B):
        full[b, 1536:2048, :] = outs[2 * b][768:1280, :] + outs[2 * b + 1][768:1280, :]
    return full


kernel.last_result = None


# revision 28
# speedup vs baseline: 1.1671x; 1.1671x over previous
"""Multi-head causal attention (B=4, N=2048, C=1024, H=16) on 8 trn2 NeuronCores.

Sharding: core c -> batch b = c//2, head-group g = c%2 (8 heads each).
Each core computes qkv projection for its heads, causal attention, and a
partial output projection over its 512 attention channels; a pair-wise
ReduceScatter(add) completes the projection, each core emitting its half of
the tokens for its batch.  Host assembles the 8 results.

v3: single software-pipelined stream.  The ScalarE exp (~157us) gates the
attention inner loop, so QKV(g+1)/proj(g-1) matmul groups are statically
interleaved into the attention kt-loops: PE fills its exp-wait gaps with
projection work and ScalarE never starves.  Causal masking moved off the
DVE into a PE accumulate-matmul (identity @ mask) into the score psum, so
exp feeds attnV directly.  Both heads' attention outputs land in one
[128,1024] psum tile: one reciprocal per pair straight from psum, one
gpsimd row-broadcast.  PSUM: scores 2x[128,1024] + ao 1x[128,1024] +
rotation 2x[128,512] = 8 banks.
"""

import os
import sys

for _p in ("/opt/trn_rl_repo",):
    if _p not in sys.path:
        sys.path.insert(0, _p)

import numpy as np

B = 4
N = 2048
C = 1024
H = 16
DK = 64
N_CORES = 8
HL = 8  # local heads per core
CL = HL * DK  # 512 local channels
PAIRS = HL // 2  # local head pairs
NT = N // 128  # 16 token tiles of 128
NQ = N // 512  # 4 query chunks of 512 (= pipeline groups)
KC = C // 128  # 8 embed contraction chunks

_nc_cache = None


def _build():
    import concourse.bass as bass
    import concourse.mybir as mybir
    import concourse.tile as tile
    from concourse import bacc
    from contextlib import ExitStack

    f32 = mybir.dt.float32
    bf16 = mybir.dt.bfloat16

    nc = bacc.Bacc("TRN2", target_bir_lowering=False, num_devices=N_CORES)

    xt_d = nc.dram_tensor("xt", [128, KC, N], bf16, kind="ExternalInput")
    w_q = nc.dram_tensor("w_q", [128, PAIRS, KC, 128], bf16, kind="ExternalInput")
    w_k = nc.dram_tensor("w_k", [128, PAIRS, KC, 128], bf16, kind="ExternalInput")
    w_v = nc.dram_tensor("w_v", [128, KC, CL], bf16, kind="ExternalInput")
    w_o = nc.dram_tensor("w_o", [128, PAIRS, C], bf16, kind="ExternalInput")
    b_q = nc.dram_tensor("b_q", [128, PAIRS], f32, kind="ExternalInput")
    b_o2 = nc.dram_tensor("b_o2", [1, C], f32, kind="ExternalInput")
    idm_d = nc.dram_tensor("idm", [128, 384], bf16, kind="ExternalInput")
    out = nc.dram_tensor("out", [768 + 512, C], bf16, kind="ExternalOutput")

    EXP = mybir.ActivationFunctionType.Exp

    with tile.TileContext(nc, pool_alloc_mode="queue") as tc, ExitStack() as st:
        # ---------- permanent pools ----------
        const = st.enter_context(tc.tile_pool(name="const", bufs=1))
        idm_sb = const.tile([128, 384], bf16)  # [0:128]=I, [128:256]=mask, [256:384]=tri
        bq_sb = const.tile([128, PAIRS], f32)
        bo_bc = const.tile([128, C], f32)
        dmy = const.tile([1, 8], f32)

        ident = idm_sb[:, 0:128]
        maskm = idm_sb[:, 128:256]
        tri_sb = idm_sb[:, 256:384]
        tri2 = bass.AP(
            tensor=idm_sb.tensor,
            offset=idm_sb.offset + 256,
            ap=[list(idm_sb.ap[0]), [0, 2], [1, 128]],
        )

        # resident weights (bf16), one contiguous tile each -> one DMA each
        w_pool = st.enter_context(tc.tile_pool(name="w", bufs=1))
        wq_all = w_pool.tile([128, PAIRS, KC, 128], bf16, tag="wq", name="wq")
        wk_all = w_pool.tile([128, PAIRS, KC, 128], bf16, tag="wk", name="wk")
        wv_all = w_pool.tile([128, KC, CL], bf16, tag="wv", name="wv")
        wo_all = w_pool.tile([128, PAIRS, C], bf16, tag="wo", name="wo")

        # persistent activations
        act = st.enter_context(tc.tile_pool(name="act", bufs=1))
        kT = [act.tile([128, N], bf16, tag=f"kT{p}", name=f"kT{p}") for p in range(PAIRS)]
        vt = act.tile([128, NT, HL, DK + 1], bf16, tag="vt", name="vt")
        # double-buffered by g-parity (written for g+1 while g is read)
        qT = [
            [act.tile([128, 512], bf16, tag=f"qT{p}_{s}", name=f"qT{p}_{s}") for p in range(PAIRS)]
            for s in range(2)
        ]
        aoT = [
            [act.tile([128, 512], bf16, tag=f"aoT{p}_{s}", name=f"aoT{p}_{s}") for p in range(PAIRS)]
            for s in range(2)
        ]

        xt_pool = st.enter_context(tc.tile_pool(name="xt", bufs=1))
        pt_pool = st.enter_context(tc.tile_pool(name="pt", bufs=8))
        nrm_pool = st.enter_context(tc.tile_pool(name="nrm", bufs=2))
        ob_pool = st.enter_context(tc.tile_pool(name="ob", bufs=3))

        ps = st.enter_context(tc.tile_pool(name="ps", bufs=1, space="PSUM"))
        dram = st.enter_context(tc.tile_pool(name="dram", bufs=1, space="DRAM"))
        rs_in = dram.tile([N, C], bf16, name="rs_in")
        rs_out = dram.tile([N // 2, C], bf16, name="rs_out")

        # ---------- startup DMAs (wv first: V(0) is the first compute) ----------
        xg = {}

        def load_xt(g):
            t = xt_pool.tile([128, KC, 512], bf16, tag="xt", bufs=2, name=f"xt{g}")
            for kc in range(KC):
                nc.sync.dma_start(
                    out=t[:, kc, :], in_=xt_d[:, kc, g * 512 : (g + 1) * 512]
                )
            xg[g] = t

        load_xt(0)
        nc.sync.dma_start(out=idm_sb, in_=idm_d[:, :])
        for j in range(4):
            nc.scalar.dma_start(
                out=wv_all[:, 2 * j : 2 * j + 2, :], in_=w_v[:, 2 * j : 2 * j + 2, :]
            )
        for p in range(PAIRS):
            nc.scalar.dma_start(out=wq_all[:, p, :, :], in_=w_q[:, p, :, :])
            nc.gpsimd.dma_start(out=wk_all[:, p, :, :], in_=w_k[:, p, :, :])
        nc.sync.dma_start(out=bq_sb, in_=b_q[:, :])
        load_xt(1)
        for p in range(PAIRS):
            nc.gpsimd.dma_start(out=wo_all[:, p, :], in_=w_o[:, p, :])
        nc.gpsimd.dma_start(out=bo_bc, in_=b_o2[0:1, :].partition_broadcast(128))

        nc.vector.memset(vt[:, :, :, DK : DK + 1], 1.0)
        # preload the exp table set while startup DMAs land
        nc.vector.memset(dmy, 0.0)
        nc.scalar.activation(dmy, dmy, EXP)

        # warm the HAM clock gate while the first DMAs land
        for w in range(16):
            warm = ps.tile([128, 512], f32, tag="rot", bufs=2, name=f"warm{w}")
            nc.tensor.matmul(
                warm[:, 0:128], ident, ident,
                start=True, stop=True, skip_group_check=True,
            )

        # ---------- filler group emitters (QKV / proj on the rotation banks) ----
        def v_group(g, i):
            xs = xg[g]
            mt = 4 * g + i
            pv = ps.tile([128, 512], f32, tag="rot", bufs=2, name=f"pv{mt}")
            mms = []
            for kc in range(KC):
                mms.append(lambda kc=kc, pv=pv, xs=xs, i=i: nc.tensor.matmul(
                    pv[:, :],
                    xs[:, kc, i * 128 : (i + 1) * 128],
                    wv_all[:, kc, :],
                    start=(kc == 0), stop=(kc == KC - 1),
                ))
            def drain(pv=pv, mt=mt):
                nc.vector.tensor_copy(
                    vt[:, mt, :, 0:DK], pv.rearrange("p (h d) -> p h d", h=HL)
                )
            mms.append(drain)
            return mms

        def q_group(g, p):
            xs = xg[g]
            pq = ps.tile([128, 512], f32, tag="rot", bufs=2, name=f"pq{g}_{p}")
            mms = []
            for kc in range(KC):
                mms.append(lambda kc=kc, pq=pq, xs=xs, p=p: nc.tensor.matmul(
                    pq[:, :], wq_all[:, p, kc, :], xs[:, kc, :],
                    start=(kc == 0), stop=(kc == KC - 1),
                ))
            def drain(pq=pq, g=g, p=p):
                nc.vector.tensor_scalar(
                    out=qT[g % 2][p][:, :], in0=pq[:, :],
                    scalar1=bq_sb[:, p : p + 1], scalar2=None,
                    op0=mybir.AluOpType.add,
                )
            mms.append(drain)
            return mms

        def k_group(g, p):
            xs = xg[g]
            pk = ps.tile([128, 512], f32, tag="rot", bufs=2, name=f"pk{g}_{p}")
            mms = []
            for kc in range(KC):
                mms.append(lambda kc=kc, pk=pk, xs=xs, p=p: nc.tensor.matmul(
                    pk[:, :], wk_all[:, p, kc, :], xs[:, kc, :],
                    start=(kc == 0), stop=(kc == KC - 1),
                ))
            def drain(pk=pk, g=g, p=p):
                nc.vector.tensor_copy(kT[p][:, g * 512 : (g + 1) * 512], pk[:, :])
            mms.append(drain)
            return mms

        def proj_group(g, grp):
            # grp in 0..7 -> (i, nn):  token tile i of chunk g, col half nn
            i, nn = divmod(grp, 2)
            mt = 4 * g + i
            pj = ps.tile([128, 512], f32, tag="rot", bufs=2, name=f"pj{mt}_{nn}")
            mms = []
            for cc in range(PAIRS):
                mms.append(lambda cc=cc, pj=pj, g=g, i=i, nn=nn: nc.tensor.matmul(
                    pj[:, :],
                    aoT[g % 2][cc][:, i * 128 : (i + 1) * 128],
                    wo_all[:, cc, nn * 512 : (nn + 1) * 512],
                    start=(cc == 0), stop=(cc == PAIRS - 1),
                ))
            def drain(pj=pj, g=g, mt=mt, nn=nn):
                ob = ob_pool.tile([128, 512], bf16, name="ob")
                nc.vector.tensor_tensor(
                    ob[:, :], pj[:, :], bo_bc[:, nn * 512 : (nn + 1) * 512],
                    mybir.AluOpType.add,
                )
                if g < NQ - 1:
                    nc.sync.dma_start(
                        out=rs_in[mt * 128 : (mt + 1) * 128, nn * 512 : (nn + 1) * 512],
                        in_=ob[:, :],
                    )
                else:
                    nc.sync.dma_start(
                        out=out[
                            768 + (mt - 12) * 128 : 768 + (mt - 11) * 128,
                            nn * 512 : (nn + 1) * 512,
                        ],
                        in_=ob[:, :],
                    )
            mms.append(drain)
            return mms

        def rs_action(g):
            g0 = g * 512
            def run():
                nc.gpsimd.collective_compute(
                    "ReduceScatter",
                    mybir.AluOpType.add,
                    replica_groups=[[0, 1], [2, 3], [4, 5], [6, 7]],
                    ins=[rs_in[g0 : g0 + 512, :].opt()],
                    outs=[rs_out[g0 // 2 : g0 // 2 + 256, :].opt()],
                )
                nc.sync.dma_start(
                    out=out[g0 // 2 : g0 // 2 + 256, :],
                    in_=rs_out[g0 // 2 : g0 // 2 + 256, :],
                )
            return [run]

        # ---------- static filler schedule per (g, p) ----------
        def filler_for(g, p):
            f = []
            if g == 0:
                # startup handles V(0), QK(0,p0/p1); spread the rest
                if p == 0:
                    f += q_group(0, 2) + k_group(0, 2)
                    f += v_group(1, 2) + v_group(1, 3)
                elif p == 1:
                    f += q_group(0, 3) + k_group(0, 3)
                elif p == 2:
                    f += q_group(1, 0) + k_group(1, 0)
                else:
                    f += q_group(1, 1) + k_group(1, 1)
                    f += v_group(1, 0) + v_group(1, 1)
                return f
            if g == 3:
                # no QKV(4): spread QK(3,*) and proj(2) as late as legal, and
                # start RS(2) one pair early so its DMA drains under p3+proj(3)
                if p == 0:
                    f += q_group(3, 1) + k_group(3, 1)
                    f += q_group(3, 2) + k_group(3, 2)
                    f += proj_group(2, 0) + proj_group(2, 1)
                elif p == 1:
                    f += q_group(3, 3) + k_group(3, 3)
                    f += proj_group(2, 2) + proj_group(2, 3) + proj_group(2, 4)
                elif p == 2:
                    f += proj_group(2, 5) + proj_group(2, 6) + proj_group(2, 7)
                    f += rs_action(2)
                return f
            # 1 <= g <= 2
            if p == 0:
                f += q_group(g, 2) + k_group(g, 2)
                f += v_group(g + 1, 2) + v_group(g + 1, 3)
                f += proj_group(g - 1, 0) + proj_group(g - 1, 1)
            elif p == 1:
                f += q_group(g, 3) + k_group(g, 3)
                f += proj_group(g - 1, 2) + proj_group(g - 1, 3) + proj_group(g - 1, 4)
            elif p == 2:
                f += q_group(g + 1, 0) + k_group(g + 1, 0)
                f += proj_group(g - 1, 5) + proj_group(g - 1, 6) + proj_group(g - 1, 7)
            else:
                f += rs_action(g - 1)
                if g < 2:
                    f += q_group(g + 1, 1) + k_group(g + 1, 1)
                f += v_group(g + 1, 0) + v_group(g + 1, 1)
            return f

        # ---------- startup compute: V(0), QK(0, p0/p1) ----------
        for i in range(4):
            for step in v_group(0, i):
                step()
        for step in q_group(0, 0) + k_group(0, 0) + q_group(0, 1) + k_group(0, 1):
            step()

        # ---------- main pipelined attention loop ----------
        for g in range(NQ):
            g0 = g * 512
            for p in range(PAIRS):
                # prefetch late enough that the DMA queue isn't head-of-line
                # blocked on the xt slot's WAR (last reader: QK(g,3) at p=1)
                if p == 2 and g + 2 < NQ:
                    load_xt(g + 2)
                n_kt = 4 * g + 4
                filler = filler_for(g, p)
                fpos = [0]

                def pump(k):
                    for _ in range(k):
                        if fpos[0] < len(filler):
                            filler[fpos[0]]()
                            fpos[0] += 1

                ao = ps.tile([128, 1024], f32, tag="ao", bufs=1, name=f"ao{g}_{p}")
                pts = {}

                def scores(kt):
                    off = 128 * (kt - 4 * g) if kt >= 4 * g else 0
                    s_t = ps.tile([128, 1024], f32, tag="s", bufs=2, name="st")
                    for h in range(2):
                        rows = slice(64 * h, 64 * h + 64)
                        nc.tensor.matmul(
                            s_t[:, 512 * h + off : 512 * h + 512],
                            kT[p][rows, kt * 128 : (kt + 1) * 128],
                            qT[g % 2][p][rows, off:512],
                            start=True, stop=True, tile_position=(64 * h, 0),
                            skip_group_check=True,
                        )
                    pt = pt_pool.tile([128, 1024], bf16, name="pt")
                    if off:
                        s4 = bass.AP(
                            tensor=s_t.tensor,
                            offset=s_t.offset + off,
                            ap=[list(s_t.ap[0]), [512, 2], [1, 512 - off]],
                        )
                        p4 = bass.AP(
                            tensor=pt.tensor,
                            offset=pt.offset + off,
                            ap=[list(pt.ap[0]), [512, 2], [1, 512 - off]],
                        )
                        nc.scalar.activation(p4, s4, EXP, scale=0.125)
                    else:
                        nc.scalar.activation(pt[:, 0:1024], s_t[:, 0:1024], EXP, scale=0.125)
                    if kt >= 4 * g:  # triangular boundary blocks, both heads
                        blk = bass.AP(
                            tensor=pt.tensor,
                            offset=pt.offset + off,
                            ap=[list(pt.ap[0]), [512, 2], [1, 128]],
                        )
                        nc.vector.tensor_tensor(blk, blk, tri2, mybir.AluOpType.mult)
                    pts[kt] = pt

                def attn_v(kt):
                    off = 128 * (kt - 4 * g) if kt >= 4 * g else 0
                    pt = pts.pop(kt)
                    for h in range(2):
                        nc.tensor.matmul(
                            ao[0:65, 512 * h + off : 512 * h + 512],
                            vt[:, kt, 2 * p + h, :],
                            pt[:, 512 * h + off : 512 * h + 512],
                            start=(kt == 0), stop=(kt == n_kt - 1),
                            skip_group_check=True,
                        )

                for kt in range(n_kt):
                    scores(kt)
                    if kt >= 2:
                        attn_v(kt - 2)
                    pump(2 if kt % 2 == 0 else 1)
                attn_v(n_kt - 2)
                attn_v(n_kt - 1)
                pump(len(filler))  # flush this pair's filler

                # softmax normalize: aoT = ao[0:64] * (1/rowsum); rowsum = ao[64]
                rs_row = nrm_pool.tile([1, 1024], f32, tag="rsr", bufs=2, name="rsr")
                nc.vector.tensor_copy(rs_row[0:1, :], ao[64:65, :])
                rcp = nrm_pool.tile([1, 1024], f32, tag="rcp", bufs=2, name="rcp")
                nc.vector.reciprocal_approx_fast(rcp[:, :], rs_row[0:1, :])
                rcpb = nrm_pool.tile([64, 1024], f32, tag="rcpb", bufs=2, name="rcpb")
                nc.gpsimd.partition_broadcast(rcpb[:, :], rcp[0:1, :], channels=64)
                for h in range(2):
                    nc.vector.tensor_tensor(
                        aoT[g % 2][p][64 * h : 64 * h + 64, :],
                        ao[0:64, 512 * h : 512 * h + 512],
                        rcpb[:, 512 * h : 512 * h + 512],
                        mybir.AluOpType.mult,
                    )

        # ---------- tail: proj(3) ----------
        for grp in range(8):
            for step in proj_group(3, grp):
                step()

    nc.compile()
    return nc


def _get_nc():
    global _nc_cache
    if _nc_cache is None:
        _nc_cache = _build()
    return _nc_cache


def kernel(x, W_qkv, b_qkv, W_o, b_o):
    import ml_dtypes
    from concourse.bass_utils import run_bass_kernel_spmd

    bf = ml_dtypes.bfloat16
    x = np.asarray(x, dtype=np.float32)
    W_qkv = np.asarray(W_qkv, dtype=np.float32)
    b_qkv = np.asarray(b_qkv, dtype=np.float32)
    W_o = np.asarray(W_o, dtype=np.float32)
    b_o = np.asarray(b_o, dtype=np.float32)

    # idm: [128, 0:128] identity; [128, 128:256] causal mask: -240 where the
    # [ktok_row, qtok_col] block entry violates j >= i (q < k).
    ident = np.eye(128, dtype=np.float32)
    m = np.where(
        np.arange(128)[None, :] >= np.arange(128)[:, None], 0.0, -240.0
    ).astype(np.float32)
    tri = np.triu(np.ones((128, 128), dtype=np.float32))
    idm = np.concatenate([ident, m, tri], axis=1)

    in_maps = []
    for c in range(N_CORES):
        b, g = divmod(c, 2)
        cs = slice(CL * g, CL * (g + 1))
        W_q_c = W_qkv[:, 0:C][:, cs]
        W_k_c = W_qkv[:, C : 2 * C][:, cs]
        W_v_c = W_qkv[:, 2 * C : 3 * C][:, cs]
        b_v_c = b_qkv[2 * C : 3 * C][cs]
        W_o_c = W_o[cs, :]
        # V-bias folds into the output bias: softmax rows sum to 1, so
        # P @ (1 b_v^T) = 1 b_v^T, and (O + 1 b_v^T) W_o = O W_o + 1 (b_v^T W_o).
        bo2 = 0.5 * b_o + b_v_c @ W_o_c
        in_maps.append(
            {
                "xt": np.ascontiguousarray(
                    x[b].T.reshape(KC, 128, N).transpose(1, 0, 2)
                ).astype(bf),
                "w_q": np.ascontiguousarray(
                    W_q_c.reshape(KC, 128, PAIRS, 128).transpose(1, 2, 0, 3)
                ).astype(bf),
                "w_k": np.ascontiguousarray(
                    W_k_c.reshape(KC, 128, PAIRS, 128).transpose(1, 2, 0, 3)
                ).astype(bf),
                "w_v": np.ascontiguousarray(
                    W_v_c.reshape(KC, 128, CL).transpose(1, 0, 2)
                ).astype(bf),
                "w_o": np.ascontiguousarray(
                    W_o_c.reshape(PAIRS, 128, C).transpose(1, 0, 2)
                ).astype(bf),
                "b_q": np.ascontiguousarray(
                    b_qkv[0:C][cs].reshape(PAIRS, 128).T
                ).astype(np.float32),
                "b_o2": np.ascontiguousarray(bo2[None, :]).astype(np.float32),
                "idm": np.ascontiguousarray(idm).astype(bf),
            }
        )

    nc = _get_nc()
    trace = bool(int(os.environ.get("BASS_KERNEL_TRACE", "0")))
    tmpdir = os.environ.get("BASS_KERNEL_TRACE_DIR") or None
    res = run_bass_kernel_spmd(
        nc, in_maps, list(range(N_CORES)), trace=trace, tmpdir=tmpdir
    )
    kernel.last_result = res

    full = np.empty((B, N, C), dtype=np.float32)
    chunks = [(0, 512), (512, 512), (1024, 512)]
    outs = [np.asarray(res.results[c]["out"], dtype=np.float32) for c in range(N_CORES)]
    for c in range(N_CORES):
        b, rank = divmod(c, 2)
        o = outs[c]
        out_r = 0
        for t0, tn in chunks:
            h = tn // 2
            full[b, t0 + rank * h : t0 + (rank + 1) * h, :] = o[out_r : out_r + h, :]
            out_r += h
    for b in range(B):
        full[b, 1536:2048, :] = outs[2 * b][768:1280, :] + outs[2 * b + 1][768:1280, :]
    return full


kernel.last_result = None


# revision 33
# speedup vs baseline: 1.1859x; 1.0160x over previous
"""Multi-head causal attention (B=4, N=2048, C=1024, H=16) on 8 trn2 NeuronCores.

Sharding: core c -> batch b = c//2, head-group g = c%2 (8 heads each).
Each core computes qkv projection for its heads, causal attention, and a
partial output projection over its 512 attention channels; a pair-wise
ReduceScatter(add) completes the projection, each core emitting its half of
the tokens for its batch.  Host assembles the 8 results.

v3: single software-pipelined stream.  The ScalarE exp (~157us) gates the
attention inner loop, so QKV(g+1)/proj(g-1) matmul groups are statically
interleaved into the attention kt-loops: PE fills its exp-wait gaps with
projection work and ScalarE never starves.  Causal masking moved off the
DVE into a PE accumulate-matmul (identity @ mask) into the score psum, so
exp feeds attnV directly.  Both heads' attention outputs land in one
[128,1024] psum tile: one reciprocal per pair straight from psum, one
gpsimd row-broadcast.  PSUM: scores 2x[128,1024] + ao 1x[128,1024] +
rotation 2x[128,512] = 8 banks.
"""

import os
import sys

for _p in ("/opt/trn_rl_repo",):
    if _p not in sys.path:
        sys.path.insert(0, _p)

import numpy as np

B = 4
N = 2048
C = 1024
H = 16
DK = 64
N_CORES = 8
HL = 8  # local heads per core
CL = HL * DK  # 512 local channels
PAIRS = HL // 2  # local head pairs
NT = N // 128  # 16 token tiles of 128
NQ = N // 512  # 4 query chunks of 512 (= pipeline groups)
KC = C // 128  # 8 embed contraction chunks

_nc_cache = None


def _build():
    import concourse.bass as bass
    import concourse.mybir as mybir
    import concourse.tile as tile
    from concourse import bacc
    from contextlib import ExitStack

    f32 = mybir.dt.float32
    bf16 = mybir.dt.bfloat16

    nc = bacc.Bacc("TRN2", target_bir_lowering=False, num_devices=N_CORES)

    xt_d = nc.dram_tensor("xt", [128, KC, N], bf16, kind="ExternalInput")
    w_q = nc.dram_tensor("w_q", [128, PAIRS, KC, 128], bf16, kind="ExternalInput")
    w_k = nc.dram_tensor("w_k", [128, PAIRS, KC, 128], bf16, kind="ExternalInput")
    w_v = nc.dram_tensor("w_v", [128, KC, CL], bf16, kind="ExternalInput")
    w_o = nc.dram_tensor("w_o", [128, PAIRS, C], bf16, kind="ExternalInput")
    b_q = nc.dram_tensor("b_q", [128, PAIRS], f32, kind="ExternalInput")
    b_o2 = nc.dram_tensor("b_o2", [1, C], f32, kind="ExternalInput")
    idm_d = nc.dram_tensor("idm", [128, 384], bf16, kind="ExternalInput")
    out = nc.dram_tensor("out", [768 + 512, C], bf16, kind="ExternalOutput")

    EXP = mybir.ActivationFunctionType.Exp

    with tile.TileContext(nc, pool_alloc_mode="queue") as tc, ExitStack() as st:
        # ---------- permanent pools ----------
        const = st.enter_context(tc.tile_pool(name="const", bufs=1))
        idm_sb = const.tile([128, 384], bf16)  # [0:128]=I, [128:256]=mask, [256:384]=tri
        bq_sb = const.tile([128, PAIRS], f32)
        bo_bc = const.tile([128, C], f32)
        dmy = const.tile([1, 8], f32)

        ident = idm_sb[:, 0:128]
        maskm = idm_sb[:, 128:256]
        tri_sb = idm_sb[:, 256:384]
        tri2 = bass.AP(
            tensor=idm_sb.tensor,
            offset=idm_sb.offset + 256,
            ap=[list(idm_sb.ap[0]), [0, 2], [1, 128]],
        )

        # resident weights (bf16), one contiguous tile each -> one DMA each
        w_pool = st.enter_context(tc.tile_pool(name="w", bufs=1))
        wq_all = w_pool.tile([128, PAIRS, KC, 128], bf16, tag="wq", name="wq")
        wk_all = w_pool.tile([128, PAIRS, KC, 128], bf16, tag="wk", name="wk")
        wv_all = w_pool.tile([128, KC, CL], bf16, tag="wv", name="wv")
        wo_all = w_pool.tile([128, PAIRS, C], bf16, tag="wo", name="wo")

        # persistent activations
        act = st.enter_context(tc.tile_pool(name="act", bufs=1))
        kT = [act.tile([128, N], bf16, tag=f"kT{p}", name=f"kT{p}") for p in range(PAIRS)]
        vt = act.tile([128, NT, HL, DK + 1], bf16, tag="vt", name="vt")
        # double-buffered by g-parity (written for g+1 while g is read)
        qT = [
            [act.tile([128, 512], bf16, tag=f"qT{p}_{s}", name=f"qT{p}_{s}") for p in range(PAIRS)]
            for s in range(2)
        ]
        aoT = [
            [act.tile([128, 512], bf16, tag=f"aoT{p}_{s}", name=f"aoT{p}_{s}") for p in range(PAIRS)]
            for s in range(2)
        ]

        xt_pool = st.enter_context(tc.tile_pool(name="xt", bufs=1))
        pt_pool = st.enter_context(tc.tile_pool(name="pt", bufs=8))
        nrm_pool = st.enter_context(tc.tile_pool(name="nrm", bufs=2))
        ob_pool = st.enter_context(tc.tile_pool(name="ob", bufs=3))

        ps = st.enter_context(tc.tile_pool(name="ps", bufs=1, space="PSUM"))
        dram = st.enter_context(tc.tile_pool(name="dram", bufs=1, space="DRAM"))
        rs_in = dram.tile([N, C], bf16, name="rs_in")
        rs_out = dram.tile([N // 2, C], bf16, name="rs_out")

        # ---------- startup DMAs (wv first: V(0) is the first compute) ----------
        xg = {}

        def load_xt(g):
            t = xt_pool.tile([128, KC, 512], bf16, tag="xt", bufs=2, name=f"xt{g}")
            for kc in range(KC):
                nc.sync.dma_start(
                    out=t[:, kc, :], in_=xt_d[:, kc, g * 512 : (g + 1) * 512]
                )
            xg[g] = t

        # first-needed data only: xt0 + wv gate V(0); wq/wk gate QK(0).
        # xt1/wo/bo are deferred so their packets don't share the DMA
        # engines with the critical startup transfers.
        load_xt(0)
        nc.sync.dma_start(out=idm_sb, in_=idm_d[:, :])
        for j in range(4):
            nc.scalar.dma_start(
                out=wv_all[:, 2 * j : 2 * j + 2, :], in_=w_v[:, 2 * j : 2 * j + 2, :]
            )
        for p in range(PAIRS):
            nc.scalar.dma_start(out=wq_all[:, p, :, :], in_=w_q[:, p, :, :])
            nc.gpsimd.dma_start(out=wk_all[:, p, :, :], in_=w_k[:, p, :, :])
        nc.sync.dma_start(out=bq_sb, in_=b_q[:, :])

        nc.vector.memset(vt[:, :, :, DK : DK + 1], 1.0)
        # preload the exp table set while startup DMAs land
        nc.vector.memset(dmy, 0.0)
        nc.scalar.activation(dmy, dmy, EXP)

        # warm the HAM clock gate while the first DMAs land
        for w in range(16):
            warm = ps.tile([128, 512], f32, tag="rot", bufs=2, name=f"warm{w}")
            nc.tensor.matmul(
                warm[:, 0:128], ident, ident,
                start=True, stop=True, skip_group_check=True,
            )

        # ---------- filler group emitters (QKV / proj on the rotation banks) ----
        def v_group(g, i):
            xs = xg[g]
            mt = 4 * g + i
            pv = ps.tile([128, 512], f32, tag="rot", bufs=2, name=f"pv{mt}")
            mms = []
            for kc in range(KC):
                mms.append(lambda kc=kc, pv=pv, xs=xs, i=i: nc.tensor.matmul(
                    pv[:, :],
                    xs[:, kc, i * 128 : (i + 1) * 128],
                    wv_all[:, kc, :],
                    start=(kc == 0), stop=(kc == KC - 1),
                ))
            def drain(pv=pv, mt=mt):
                nc.vector.tensor_copy(
                    vt[:, mt, :, 0:DK], pv.rearrange("p (h d) -> p h d", h=HL)
                )
            mms.append(drain)
            return mms

        def q_group(g, p):
            xs = xg[g]
            pq = ps.tile([128, 512], f32, tag="rot", bufs=2, name=f"pq{g}_{p}")
            mms = []
            for kc in range(KC):
                mms.append(lambda kc=kc, pq=pq, xs=xs, p=p: nc.tensor.matmul(
                    pq[:, :], wq_all[:, p, kc, :], xs[:, kc, :],
                    start=(kc == 0), stop=(kc == KC - 1),
                ))
            def drain(pq=pq, g=g, p=p):
                nc.vector.tensor_scalar(
                    out=qT[g % 2][p][:, :], in0=pq[:, :],
                    scalar1=bq_sb[:, p : p + 1], scalar2=None,
                    op0=mybir.AluOpType.add,
                )
            mms.append(drain)
            return mms

        def k_group(g, p):
            xs = xg[g]
            pk = ps.tile([128, 512], f32, tag="rot", bufs=2, name=f"pk{g}_{p}")
            mms = []
            for kc in range(KC):
                mms.append(lambda kc=kc, pk=pk, xs=xs, p=p: nc.tensor.matmul(
                    pk[:, :], wk_all[:, p, kc, :], xs[:, kc, :],
                    start=(kc == 0), stop=(kc == KC - 1),
                ))
            def drain(pk=pk, g=g, p=p):
                nc.vector.tensor_copy(kT[p][:, g * 512 : (g + 1) * 512], pk[:, :])
            mms.append(drain)
            return mms

        def proj_group(g, grp):
            # grp in 0..7 -> (i, nn):  token tile i of chunk g, col half nn
            i, nn = divmod(grp, 2)
            mt = 4 * g + i
            pj = ps.tile([128, 512], f32, tag="rot", bufs=2, name=f"pj{mt}_{nn}")
            mms = []
            for cc in range(PAIRS):
                mms.append(lambda cc=cc, pj=pj, g=g, i=i, nn=nn: nc.tensor.matmul(
                    pj[:, :],
                    aoT[g % 2][cc][:, i * 128 : (i + 1) * 128],
                    wo_all[:, cc, nn * 512 : (nn + 1) * 512],
                    start=(cc == 0), stop=(cc == PAIRS - 1),
                ))
            def drain(pj=pj, g=g, mt=mt, nn=nn):
                ob = ob_pool.tile([128, 512], bf16, name="ob")
                nc.vector.tensor_tensor(
                    ob[:, :], pj[:, :], bo_bc[:, nn * 512 : (nn + 1) * 512],
                    mybir.AluOpType.add,
                )
                if g < NQ - 1:
                    nc.sync.dma_start(
                        out=rs_in[mt * 128 : (mt + 1) * 128, nn * 512 : (nn + 1) * 512],
                        in_=ob[:, :],
                    )
                else:
                    nc.sync.dma_start(
                        out=out[
                            768 + (mt - 12) * 128 : 768 + (mt - 11) * 128,
                            nn * 512 : (nn + 1) * 512,
                        ],
                        in_=ob[:, :],
                    )
            mms.append(drain)
            return mms

        def rs_action(g):
            g0 = g * 512
            def run():
                nc.gpsimd.collective_compute(
                    "ReduceScatter",
                    mybir.AluOpType.add,
                    replica_groups=[[0, 1], [2, 3], [4, 5], [6, 7]],
                    ins=[rs_in[g0 : g0 + 512, :].opt()],
                    outs=[rs_out[g0 // 2 : g0 // 2 + 256, :].opt()],
                )
                nc.sync.dma_start(
                    out=out[g0 // 2 : g0 // 2 + 256, :],
                    in_=rs_out[g0 // 2 : g0 // 2 + 256, :],
                )
            return [run]

        # ---------- static filler schedule per (g, p) ----------
        def filler_for(g, p):
            f = []
            if g == 0:
                # startup handles V(0), QK(0,p0/p1); spread the rest
                if p == 0:
                    f += q_group(0, 2) + k_group(0, 2)
                    f += v_group(1, 2) + v_group(1, 3)
                elif p == 1:
                    f += q_group(0, 3) + k_group(0, 3)
                elif p == 2:
                    f += q_group(1, 0) + k_group(1, 0)
                else:
                    f += q_group(1, 1) + k_group(1, 1)
                    f += v_group(1, 0) + v_group(1, 1)
                return f
            if g == 3:
                # no QKV(4): spread QK(3,*) and proj(2) as late as legal, and
                # start RS(2) one pair early so its DMA drains under p3+proj(3)
                if p == 0:
                    f += q_group(3, 1) + k_group(3, 1)
                    f += q_group(3, 2) + k_group(3, 2)
                    f += proj_group(2, 0) + proj_group(2, 1)
                elif p == 1:
                    f += q_group(3, 3) + k_group(3, 3)
                    f += proj_group(2, 2) + proj_group(2, 3) + proj_group(2, 4)
                elif p == 2:
                    f += proj_group(2, 5) + proj_group(2, 6) + proj_group(2, 7)
                    f += rs_action(2)
                else:
                    # proj(3) groups 0-1: cc0-2 partials overlap this pair's
                    # exp-bound window (rotation banks only; s/ao banks would
                    # deadlock against the score/ao rotation)
                    for grp in (0, 1):
                        steps = proj_group(3, grp)
                        f += steps[:3]
                        tail_finals.extend(steps[3:])
                return f
            # 1 <= g <= 2
            if p == 0:
                f += q_group(g, 2) + k_group(g, 2)
                f += v_group(g + 1, 2) + v_group(g + 1, 3)
                f += proj_group(g - 1, 0) + proj_group(g - 1, 1)
            elif p == 1:
                f += q_group(g, 3) + k_group(g, 3)
                f += proj_group(g - 1, 2) + proj_group(g - 1, 3) + proj_group(g - 1, 4)
            elif p == 2:
                f += q_group(g + 1, 0) + k_group(g + 1, 0)
                f += proj_group(g - 1, 5) + proj_group(g - 1, 6) + proj_group(g - 1, 7)
            else:
                f += rs_action(g - 1)
                if g < 2:
                    f += q_group(g + 1, 1) + k_group(g + 1, 1)
                f += v_group(g + 1, 0) + v_group(g + 1, 1)
            return f

        # ---------- startup compute: V(0), QK(0, p0/p1) ----------
        for i in range(4):
            for step in v_group(0, i):
                step()
        for step in q_group(0, 0) + k_group(0, 0) + q_group(0, 1) + k_group(0, 1):
            step()

        # second-wave DMAs (needed from attention(1) / proj(0) onward)
        load_xt(1)
        for p in range(PAIRS):
            nc.gpsimd.dma_start(out=wo_all[:, p, :], in_=w_o[:, p, :])
        nc.gpsimd.dma_start(out=bo_bc, in_=b_o2[0:1, :].partition_broadcast(128))

        tail_finals = []

        # ---------- main pipelined attention loop ----------
        for g in range(NQ):
            g0 = g * 512
            for p in range(PAIRS):
                # prefetch late enough that the DMA queue isn't head-of-line
                # blocked on the xt slot's WAR (last reader: QK(g,3) at p=1)
                if p == 2 and g + 2 < NQ:
                    load_xt(g + 2)
                n_kt = 4 * g + 4
                filler = filler_for(g, p)
                fpos = [0]

                def pump(k):
                    for _ in range(k):
                        if fpos[0] < len(filler):
                            filler[fpos[0]]()
                            fpos[0] += 1

                ao = ps.tile([128, 1024], f32, tag="ao", bufs=1, name=f"ao{g}_{p}")
                pts = {}

                def scores(kt):
                    off = 128 * (kt - 4 * g) if kt >= 4 * g else 0
                    s_t = ps.tile([128, 1024], f32, tag="s", bufs=2, name="st")
                    for h in range(2):
                        rows = slice(64 * h, 64 * h + 64)
                        nc.tensor.matmul(
                            s_t[:, 512 * h + off : 512 * h + 512],
                            kT[p][rows, kt * 128 : (kt + 1) * 128],
                            qT[g % 2][p][rows, off:512],
                            start=True, stop=True, tile_position=(64 * h, 0),
                            skip_group_check=True,
                        )
                    pt = pt_pool.tile([128, 1024], bf16, name="pt")
                    if off:
                        s4 = bass.AP(
                            tensor=s_t.tensor,
                            offset=s_t.offset + off,
                            ap=[list(s_t.ap[0]), [512, 2], [1, 512 - off]],
                        )
                        p4 = bass.AP(
                            tensor=pt.tensor,
                            offset=pt.offset + off,
                            ap=[list(pt.ap[0]), [512, 2], [1, 512 - off]],
                        )
                        nc.scalar.activation(p4, s4, EXP, scale=0.125)
                    else:
                        nc.scalar.activation(pt[:, 0:1024], s_t[:, 0:1024], EXP, scale=0.125)
                    if kt >= 4 * g:  # triangular boundary blocks, both heads
                        blk = bass.AP(
                            tensor=pt.tensor,
                            offset=pt.offset + off,
                            ap=[list(pt.ap[0]), [512, 2], [1, 128]],
                        )
                        nc.vector.tensor_tensor(blk, blk, tri2, mybir.AluOpType.mult)
                    pts[kt] = pt

                def attn_v(kt):
                    off = 128 * (kt - 4 * g) if kt >= 4 * g else 0
                    pt = pts.pop(kt)
                    for h in range(2):
                        nc.tensor.matmul(
                            ao[0:65, 512 * h + off : 512 * h + 512],
                            vt[:, kt, 2 * p + h, :],
                            pt[:, 512 * h + off : 512 * h + 512],
                            start=(kt == 0), stop=(kt == n_kt - 1),
                            skip_group_check=True,
                        )

                for kt in range(n_kt):
                    scores(kt)
                    if kt >= 2:
                        attn_v(kt - 2)
                    pump(2 if kt % 2 == 0 else 1)
                attn_v(n_kt - 2)
                attn_v(n_kt - 1)
                pump(len(filler))  # flush this pair's filler

                # softmax normalize: aoT = ao[0:64] * (1/rowsum); rowsum = ao[64]
                rs_row = nrm_pool.tile([1, 1024], f32, tag="rsr", bufs=2, name="rsr")
                nc.vector.tensor_copy(rs_row[0:1, :], ao[64:65, :])
                rcp = nrm_pool.tile([1, 1024], f32, tag="rcp", bufs=2, name="rcp")
                nc.vector.reciprocal_approx_fast(rcp[:, :], rs_row[0:1, :])
                rcpb = nrm_pool.tile([64, 1024], f32, tag="rcpb", bufs=2, name="rcpb")
                nc.gpsimd.partition_broadcast(rcpb[:, :], rcp[0:1, :], channels=64)
                for h in range(2):
                    nc.vector.tensor_tensor(
                        aoT[g % 2][p][64 * h : 64 * h + 64, :],
                        ao[0:64, 512 * h : 512 * h + 512],
                        rcpb[:, 512 * h : 512 * h + 512],
                        mybir.AluOpType.mult,
                    )

        # ---------- tail: finish proj(3) ----------
        for step in tail_finals:
            step()
        for grp in range(2, 8):
            for step in proj_group(3, grp):
                step()

    nc.compile()
    return nc


def _get_nc():
    global _nc_cache
    if _nc_cache is None:
        _nc_cache = _build()
    return _nc_cache


def kernel(x, W_qkv, b_qkv, W_o, b_o):
    import ml_dtypes
    from concourse.bass_utils import run_bass_kernel_spmd

    bf = ml_dtypes.bfloat16
    x = np.asarray(x, dtype=np.float32)
    W_qkv = np.asarray(W_qkv, dtype=np.float32)
    b_qkv = np.asarray(b_qkv, dtype=np.float32)
    W_o = np.asarray(W_o, dtype=np.float32)
    b_o = np.asarray(b_o, dtype=np.float32)

    # idm: [128, 0:128] identity; [128, 128:256] causal mask: -240 where the
    # [ktok_row, qtok_col] block entry violates j >= i (q < k).
    ident = np.eye(128, dtype=np.float32)
    m = np.where(
        np.arange(128)[None, :] >= np.arange(128)[:, None], 0.0, -240.0
    ).astype(np.float32)
    tri = np.triu(np.ones((128, 128), dtype=np.float32))
    idm = np.concatenate([ident, m, tri], axis=1)

    in_maps = []
    for c in range(N_CORES):
        b, g = divmod(c, 2)
        cs = slice(CL * g, CL * (g + 1))
        W_q_c = W_qkv[:, 0:C][:, cs]
        W_k_c = W_qkv[:, C : 2 * C][:, cs]
        W_v_c = W_qkv[:, 2 * C : 3 * C][:, cs]
        b_v_c = b_qkv[2 * C : 3 * C][cs]
        W_o_c = W_o[cs, :]
        # V-bias folds into the output bias: softmax rows sum to 1, so
        # P @ (1 b_v^T) = 1 b_v^T, and (O + 1 b_v^T) W_o = O W_o + 1 (b_v^T W_o).
        bo2 = 0.5 * b_o + b_v_c @ W_o_c
        in_maps.append(
            {
                "xt": np.ascontiguousarray(
                    x[b].T.reshape(KC, 128, N).transpose(1, 0, 2)
                ).astype(bf),
                "w_q": np.ascontiguousarray(
                    W_q_c.reshape(KC, 128, PAIRS, 128).transpose(1, 2, 0, 3)
                ).astype(bf),
                "w_k": np.ascontiguousarray(
                    W_k_c.reshape(KC, 128, PAIRS, 128).transpose(1, 2, 0, 3)
                ).astype(bf),
                "w_v": np.ascontiguousarray(
                    W_v_c.reshape(KC, 128, CL).transpose(1, 0, 2)
                ).astype(bf),
                "w_o": np.ascontiguousarray(
                    W_o_c.reshape(PAIRS, 128, C).transpose(1, 0, 2)
                ).astype(bf),
                "b_q": np.ascontiguousarray(
                    b_qkv[0:C][cs].reshape(PAIRS, 128).T
                ).astype(np.float32),
                "b_o2": np.ascontiguousarray(bo2[None, :]).astype(np.float32),
                "idm": np.ascontiguousarray(idm).astype(bf),
            }
        )

    nc = _get_nc()
    trace = bool(int(os.environ.get("BASS_KERNEL_TRACE", "0")))
    tmpdir = os.environ.get("BASS_KERNEL_TRACE_DIR") or None
    res = run_bass_kernel_spmd(
        nc, in_maps, list(range(N_CORES)), trace=trace, tmpdir=tmpdir
    )
    kernel.last_result = res

    full = np.empty((B, N, C), dtype=np.float32)
    chunks = [(0, 512), (512, 512), (1024, 512)]
    outs = [np.asarray(res.results[c]["out"], dtype=np.float32) for c in range(N_CORES)]
    for c in range(N_CORES):
        b, rank = divmod(c, 2)
        o = outs[c]
        out_r = 0
        for t0, tn in chunks:
            h = tn // 2
            full[b, t0 + rank * h : t0 + (rank + 1) * h, :] = o[out_r : out_r + h, :]
            out_r += h
    for b in range(B):
        full[b, 1536:2048, :] = outs[2 * b][768:1280, :] + outs[2 * b + 1][768:1280, :]
    return full


kernel.last_result = None


# revision 45
# speedup vs baseline: 1.1955x; 1.0081x over previous
"""Multi-head causal attention (B=4, N=2048, C=1024, H=16) on 8 trn2 NeuronCores.

Sharding: core c -> batch b = c//2, head-group g = c%2 (8 heads each).
Each core computes qkv projection for its heads, causal attention, and a
partial output projection over its 512 attention channels; a pair-wise
ReduceScatter(add) completes the projection, each core emitting its half of
the tokens for its batch.  Host assembles the 8 results.

v3: single software-pipelined stream.  The ScalarE exp (~157us) gates the
attention inner loop, so QKV(g+1)/proj(g-1) matmul groups are statically
interleaved into the attention kt-loops: PE fills its exp-wait gaps with
projection work and ScalarE never starves.  Causal masking moved off the
DVE into a PE accumulate-matmul (identity @ mask) into the score psum, so
exp feeds attnV directly.  Both heads' attention outputs land in one
[128,1024] psum tile: one reciprocal per pair straight from psum, one
gpsimd row-broadcast.  PSUM: scores 2x[128,1024] + ao 1x[128,1024] +
rotation 2x[128,512] = 8 banks.
"""

import os
import sys

for _p in ("/opt/trn_rl_repo",):
    if _p not in sys.path:
        sys.path.insert(0, _p)

import numpy as np

B = 4
N = 2048
C = 1024
H = 16
DK = 64
N_CORES = 8
HL = 8  # local heads per core
CL = HL * DK  # 512 local channels
PAIRS = HL // 2  # local head pairs
NT = N // 128  # 16 token tiles of 128
NQ = N // 512  # 4 query chunks of 512 (= pipeline groups)
KC = C // 128  # 8 embed contraction chunks

_nc_cache = None


def _build():
    import concourse.bass as bass
    import concourse.mybir as mybir
    import concourse.tile as tile
    from concourse import bacc
    from contextlib import ExitStack

    f32 = mybir.dt.float32
    bf16 = mybir.dt.bfloat16

    nc = bacc.Bacc("TRN2", target_bir_lowering=False, num_devices=N_CORES)

    xt_d = nc.dram_tensor("xt", [128, KC, N], bf16, kind="ExternalInput")
    w_q = nc.dram_tensor("w_q", [128, PAIRS, KC, 128], bf16, kind="ExternalInput")
    w_k = nc.dram_tensor("w_k", [128, PAIRS, KC, 128], bf16, kind="ExternalInput")
    w_v = nc.dram_tensor("w_v", [128, KC, CL], bf16, kind="ExternalInput")
    w_o = nc.dram_tensor("w_o", [128, PAIRS, C], bf16, kind="ExternalInput")
    b_q = nc.dram_tensor("b_q", [128, PAIRS], f32, kind="ExternalInput")
    b_o2 = nc.dram_tensor("b_o2", [1, C], f32, kind="ExternalInput")
    idm_d = nc.dram_tensor("idm", [128, 384], bf16, kind="ExternalInput")
    out = nc.dram_tensor("out", [768 + 512, C], bf16, kind="ExternalOutput")

    EXP = mybir.ActivationFunctionType.Exp

    with tile.TileContext(nc, pool_alloc_mode="queue") as tc, ExitStack() as st:
        # ---------- permanent pools ----------
        const = st.enter_context(tc.tile_pool(name="const", bufs=1))
        idm_sb = const.tile([128, 384], bf16)  # [0:128]=I, [128:256]=mask, [256:384]=tri
        bq_sb = const.tile([128, PAIRS], f32)
        bo_bc = const.tile([128, C], f32)
        dmy = const.tile([1, 8], f32)
        garb = const.tile([128, 512], bf16)  # memset warmup fodder (no DMA dep)

        ident = idm_sb[:, 0:128]
        maskm = idm_sb[:, 128:256]
        tri_sb = idm_sb[:, 256:384]
        tri2 = bass.AP(
            tensor=idm_sb.tensor,
            offset=idm_sb.offset + 256,
            ap=[list(idm_sb.ap[0]), [0, 2], [1, 128]],
        )

        # resident weights (bf16); tiles kept fine-grained so compute gates on
        # only the chunk it reads (whole-tile deps were a 10us startup tax)
        w_pool = st.enter_context(tc.tile_pool(name="w", bufs=1))
        wq_sb = [w_pool.tile([128, KC, 128], bf16, tag=f"wq{p}", name=f"wq{p}") for p in range(PAIRS)]
        wk_sb = [w_pool.tile([128, KC, 128], bf16, tag=f"wk{p}", name=f"wk{p}") for p in range(PAIRS)]
        wv_sb = [w_pool.tile([128, 2, CL], bf16, tag=f"wv{j}", name=f"wv{j}") for j in range(KC // 2)]
        wo_sb = [w_pool.tile([128, C], bf16, tag=f"wo{p}", name=f"wo{p}") for p in range(PAIRS)]

        # persistent activations
        act = st.enter_context(tc.tile_pool(name="act", bufs=1))
        kT = [act.tile([128, N], bf16, tag=f"kT{p}", name=f"kT{p}") for p in range(PAIRS)]
        vt = act.tile([128, NT, HL, DK + 1], bf16, tag="vt", name="vt")
        # double-buffered by g-parity (written for g+1 while g is read)
        qT = [
            [act.tile([128, 512], bf16, tag=f"qT{p}_{s}", name=f"qT{p}_{s}") for p in range(PAIRS)]
            for s in range(2)
        ]
        aoT = [
            [act.tile([128, 512], bf16, tag=f"aoT{p}_{s}", name=f"aoT{p}_{s}") for p in range(PAIRS)]
            for s in range(2)
        ]

        xt_pool = st.enter_context(tc.tile_pool(name="xt", bufs=1))
        pt_pool = st.enter_context(tc.tile_pool(name="pt", bufs=8))
        nrm_pool = st.enter_context(tc.tile_pool(name="nrm", bufs=2))
        ob_pool = st.enter_context(tc.tile_pool(name="ob", bufs=3))

        ps = st.enter_context(tc.tile_pool(name="ps", bufs=1, space="PSUM"))
        dram = st.enter_context(tc.tile_pool(name="dram", bufs=1, space="DRAM"))
        rs_in = dram.tile([N, C], bf16, name="rs_in")
        rs_out = dram.tile([N // 2, C], bf16, name="rs_out")

        # ---------- startup DMAs (wv first: V(0) is the first compute) ----------
        xg = {}

        def load_xt(g, eng=None):
            e = eng if eng is not None else nc.sync
            tiles = [
                xt_pool.tile([128, 2, 512], bf16, tag=f"xt{j}", bufs=2, name=f"xt{g}_{j}")
                for j in range(KC // 2)
            ]
            for j in range(KC // 2):
                e.dma_start(
                    out=tiles[j], in_=xt_d[:, 2 * j : 2 * j + 2, g * 512 : (g + 1) * 512]
                )
            xg[g] = tiles

        # first-needed data only: xt0 + wv gate V(0); wq/wk gate QK(0).
        # xt1/wo/bo fire after the first normalize's gpsimd broadcast (their
        # queue position gates them), so their packets don't share the DMA
        # engines with the critical startup transfers.
        load_xt(0)
        nc.sync.dma_start(out=idm_sb, in_=idm_d[:, :])
        for j in range(KC // 2):
            nc.scalar.dma_start(out=wv_sb[j], in_=w_v[:, 2 * j : 2 * j + 2, :])
        for p in range(PAIRS):
            nc.scalar.dma_start(out=wq_sb[p], in_=w_q[:, p, :, :])
            nc.gpsimd.dma_start(out=wk_sb[p], in_=w_k[:, p, :, :])
        nc.sync.dma_start(out=bq_sb, in_=b_q[:, :])

        nc.vector.memset(vt[:, :, :, DK : DK + 1], 1.0)
        # preload the exp table set while startup DMAs land
        nc.vector.memset(dmy, 0.0)
        nc.scalar.activation(dmy, dmy, EXP)

        # warm the HAM clock gate from ~1us (garb has no DMA dependency) and
        # keep the PE busy-window alive until the startup DMAs land
        nc.vector.memset(garb, 0.0)
        for w in range(56):
            warm = ps.tile([128, 512], f32, tag="rot", bufs=2, name=f"warm{w}")
            nc.tensor.matmul(
                warm[:, :], garb[:, 0:128], garb[:, :],
                start=True, stop=True, skip_group_check=True,
            )

        # ---------- filler group emitters (QKV / proj on the rotation banks) ----
        def v_group(g, i):
            xs = xg[g]
            mt = 4 * g + i
            pv = ps.tile([128, 512], f32, tag="rot", bufs=2, name=f"pv{mt}")
            mms = []
            for kc in range(KC):
                mms.append(lambda kc=kc, pv=pv, xs=xs, i=i: nc.tensor.matmul(
                    pv[:, :],
                    xs[kc >> 1][:, kc & 1, i * 128 : (i + 1) * 128],
                    wv_sb[kc >> 1][:, kc & 1, :],
                    start=(kc == 0), stop=(kc == KC - 1),
                ))
            def drain(pv=pv, mt=mt):
                nc.vector.tensor_copy(
                    vt[:, mt, :, 0:DK], pv.rearrange("p (h d) -> p h d", h=HL)
                )
            mms.append(drain)
            return mms

        def q_group(g, p):
            xs = xg[g]
            pq = ps.tile([128, 512], f32, tag="rot", bufs=2, name=f"pq{g}_{p}")
            mms = []
            for kc in range(KC):
                mms.append(lambda kc=kc, pq=pq, xs=xs, p=p: nc.tensor.matmul(
                    pq[:, :], wq_sb[p][:, kc, :], xs[kc >> 1][:, kc & 1, :],
                    start=(kc == 0), stop=(kc == KC - 1),
                ))
            def drain(pq=pq, g=g, p=p):
                nc.vector.tensor_scalar(
                    out=qT[g % 2][p][:, :], in0=pq[:, :],
                    scalar1=bq_sb[:, p : p + 1], scalar2=None,
                    op0=mybir.AluOpType.add,
                )
            mms.append(drain)
            return mms

        def k_group(g, p):
            xs = xg[g]
            pk = ps.tile([128, 512], f32, tag="rot", bufs=2, name=f"pk{g}_{p}")
            mms = []
            for kc in range(KC):
                mms.append(lambda kc=kc, pk=pk, xs=xs, p=p: nc.tensor.matmul(
                    pk[:, :], wk_sb[p][:, kc, :], xs[kc >> 1][:, kc & 1, :],
                    start=(kc == 0), stop=(kc == KC - 1),
                ))
            def drain(pk=pk, g=g, p=p):
                nc.vector.tensor_copy(kT[p][:, g * 512 : (g + 1) * 512], pk[:, :])
            mms.append(drain)
            return mms

        def proj_group(g, grp):
            # grp in 0..7 -> (i, nn):  token tile i of chunk g, col half nn
            i, nn = divmod(grp, 2)
            mt = 4 * g + i
            pj = ps.tile([128, 512], f32, tag="rot", bufs=2, name=f"pj{mt}_{nn}")
            mms = []
            for cc in range(PAIRS):
                mms.append(lambda cc=cc, pj=pj, g=g, i=i, nn=nn: nc.tensor.matmul(
                    pj[:, :],
                    aoT[g % 2][cc][:, i * 128 : (i + 1) * 128],
                    wo_sb[cc][:, nn * 512 : (nn + 1) * 512],
                    start=(cc == 0), stop=(cc == PAIRS - 1),
                ))
            def drain(pj=pj, g=g, mt=mt, nn=nn):
                ob = ob_pool.tile([128, 512], bf16, name="ob")
                nc.vector.tensor_tensor(
                    ob[:, :], pj[:, :], bo_bc[:, nn * 512 : (nn + 1) * 512],
                    mybir.AluOpType.add,
                )
                if g < NQ - 1:
                    nc.sync.dma_start(
                        out=rs_in[mt * 128 : (mt + 1) * 128, nn * 512 : (nn + 1) * 512],
                        in_=ob[:, :],
                    )
                else:
                    nc.sync.dma_start(
                        out=out[
                            768 + (mt - 12) * 128 : 768 + (mt - 11) * 128,
                            nn * 512 : (nn + 1) * 512,
                        ],
                        in_=ob[:, :],
                    )
            mms.append(drain)
            return mms

        def rs_action(g):
            g0 = g * 512
            def run():
                nc.gpsimd.collective_compute(
                    "ReduceScatter",
                    mybir.AluOpType.add,
                    replica_groups=[[0, 1], [2, 3], [4, 5], [6, 7]],
                    ins=[rs_in[g0 : g0 + 512, :].opt()],
                    outs=[rs_out[g0 // 2 : g0 // 2 + 256, :].opt()],
                )
                nc.sync.dma_start(
                    out=out[g0 // 2 : g0 // 2 + 256, :],
                    in_=rs_out[g0 // 2 : g0 // 2 + 256, :],
                )
            return [run]

        # ---------- static filler schedule per (g, p) ----------
        def filler_for(g, p):
            f = []
            if g == 0:
                # startup handles V(0), QK(0,p0/p1); QKV(1) waits for xt1
                # (second wave, emitted after normalize(0,0))
                if p == 0:
                    f += q_group(0, 2) + k_group(0, 2)
                elif p == 1:
                    f += q_group(0, 3) + k_group(0, 3)
                elif p == 2:
                    f += q_group(1, 0) + k_group(1, 0)
                    f += v_group(1, 2) + v_group(1, 3)
                else:
                    f += q_group(1, 1) + k_group(1, 1)
                    f += v_group(1, 0) + v_group(1, 1)
                return f
            if g == 3:
                # no QKV(4): spread QK(3,*) and proj(2) as late as legal, and
                # start RS(2) one pair early so its DMA drains under p3+proj(3)
                if p == 0:
                    f += q_group(3, 1) + k_group(3, 1)
                    f += q_group(3, 2) + k_group(3, 2)
                    f += proj_group(2, 0) + proj_group(2, 1)
                elif p == 1:
                    f += q_group(3, 3) + k_group(3, 3)
                    f += proj_group(2, 2) + proj_group(2, 3) + proj_group(2, 4)
                elif p == 2:
                    f += proj_group(2, 5) + proj_group(2, 6) + proj_group(2, 7)
                    f += rs_action(2)
                else:
                    # proj(3) groups 0-1: cc0-2 partials overlap this pair's
                    # exp-bound window (rotation banks only; s/ao banks would
                    # deadlock against the score/ao rotation)
                    for grp in (0, 1):
                        steps = proj_group(3, grp)
                        f += steps[:3]
                        tail_finals.extend(steps[3:])
                return f
            # 1 <= g <= 2
            if p == 0:
                f += q_group(g, 2) + k_group(g, 2)
                f += v_group(g + 1, 2) + v_group(g + 1, 3)
                f += proj_group(g - 1, 0) + proj_group(g - 1, 1)
            elif p == 1:
                f += q_group(g, 3) + k_group(g, 3)
                f += proj_group(g - 1, 2) + proj_group(g - 1, 3) + proj_group(g - 1, 4)
            elif p == 2:
                f += q_group(g + 1, 0) + k_group(g + 1, 0)
                f += proj_group(g - 1, 5) + proj_group(g - 1, 6) + proj_group(g - 1, 7)
            else:
                f += rs_action(g - 1)
                if g < 2:
                    f += q_group(g + 1, 1) + k_group(g + 1, 1)
                f += v_group(g + 1, 0) + v_group(g + 1, 1)
            return f

        # ---------- startup compute: V(0), QK(0, p0/p1) ----------
        for i in range(4):
            for step in v_group(0, i):
                step()
        for step in q_group(0, 0) + k_group(0, 0) + q_group(0, 1) + k_group(0, 1):
            step()

        tail_finals = []

        # ---------- main pipelined attention loop ----------
        for g in range(NQ):
            g0 = g * 512
            for p in range(PAIRS):
                # prefetch late enough that the DMA queue isn't head-of-line
                # blocked on the xt slot's WAR (last reader: QK(g,3) at p=1)
                if p == 2 and g + 2 < NQ:
                    load_xt(g + 2)
                n_kt = 4 * g + 4
                filler = filler_for(g, p)
                fpos = [0]

                def pump(k):
                    for _ in range(k):
                        if fpos[0] < len(filler):
                            filler[fpos[0]]()
                            fpos[0] += 1

                ao = ps.tile([128, 1024], f32, tag="ao", bufs=1, name=f"ao{g}_{p}")
                pts = {}

                def scores(kt):
                    off = 128 * (kt - 4 * g) if kt >= 4 * g else 0
                    s_t = ps.tile([128, 1024], f32, tag="s", bufs=2, name="st")
                    for h in range(2):
                        rows = slice(64 * h, 64 * h + 64)
                        nc.tensor.matmul(
                            s_t[:, 512 * h + off : 512 * h + 512],
                            kT[p][rows, kt * 128 : (kt + 1) * 128],
                            qT[g % 2][p][rows, off:512],
                            start=True, stop=True, tile_position=(64 * h, 0),
                            skip_group_check=True,
                        )
                    pt = pt_pool.tile([128, 1024], bf16, name="pt")
                    if off:
                        s4 = bass.AP(
                            tensor=s_t.tensor,
                            offset=s_t.offset + off,
                            ap=[list(s_t.ap[0]), [512, 2], [1, 512 - off]],
                        )
                        p4 = bass.AP(
                            tensor=pt.tensor,
                            offset=pt.offset + off,
                            ap=[list(pt.ap[0]), [512, 2], [1, 512 - off]],
                        )
                        nc.scalar.activation(p4, s4, EXP, scale=0.125)
                    else:
                        nc.scalar.activation(pt[:, 0:1024], s_t[:, 0:1024], EXP, scale=0.125)
                    if kt >= 4 * g:  # triangular boundary blocks, both heads
                        blk = bass.AP(
                            tensor=pt.tensor,
                            offset=pt.offset + off,
                            ap=[list(pt.ap[0]), [512, 2], [1, 128]],
                        )
                        nc.vector.tensor_tensor(blk, blk, tri2, mybir.AluOpType.mult)
                    pts[kt] = pt

                def attn_v(kt):
                    off = 128 * (kt - 4 * g) if kt >= 4 * g else 0
                    pt = pts.pop(kt)
                    for h in range(2):
                        nc.tensor.matmul(
                            ao[0:65, 512 * h + off : 512 * h + 512],
                            vt[:, kt, 2 * p + h, :],
                            pt[:, 512 * h + off : 512 * h + 512],
                            start=(kt == 0), stop=(kt == n_kt - 1),
                            skip_group_check=True,
                        )

                for kt in range(n_kt):
                    scores(kt)
                    if kt >= 2:
                        attn_v(kt - 2)
                    pump(2 if kt % 2 == 0 else 1)
                attn_v(n_kt - 2)
                attn_v(n_kt - 1)
                pump(len(filler))  # flush this pair's filler

                # softmax normalize: aoT = ao[0:64] * (1/rowsum); rowsum = ao[64]
                rs_row = nrm_pool.tile([1, 1024], f32, tag="rsr", bufs=2, name="rsr")
                nc.vector.tensor_copy(rs_row[0:1, :], ao[64:65, :])
                rcp = nrm_pool.tile([1, 1024], f32, tag="rcp", bufs=2, name="rcp")
                nc.vector.reciprocal_approx_fast(rcp[:, :], rs_row[0:1, :])
                rcpb = nrm_pool.tile([64, 1024], f32, tag="rcpb", bufs=2, name="rcpb")
                nc.gpsimd.partition_broadcast(rcpb[:, :], rcp[0:1, :], channels=64)
                for h in range(2):
                    nc.vector.tensor_tensor(
                        aoT[g % 2][p][64 * h : 64 * h + 64, :],
                        ao[0:64, 512 * h : 512 * h + 512],
                        rcpb[:, 512 * h : 512 * h + 512],
                        mybir.AluOpType.mult,
                    )

                if g == 0 and p == 0:
                    # second-wave DMAs: queued on gpsimd behind the broadcast
                    # above, so they fire only once startup transfers are done
                    load_xt(1, eng=nc.gpsimd)
                    for pp in range(PAIRS):
                        nc.gpsimd.dma_start(out=wo_sb[pp], in_=w_o[:, pp, :])
                    nc.gpsimd.dma_start(
                        out=bo_bc, in_=b_o2[0:1, :].partition_broadcast(128)
                    )

        # ---------- tail: finish proj(3) ----------
        for step in tail_finals:
            step()
        for grp in range(2, 8):
            for step in proj_group(3, grp):
                step()

    nc.compile()
    return nc


def _get_nc():
    global _nc_cache
    if _nc_cache is None:
        _nc_cache = _build()
    return _nc_cache


def kernel(x, W_qkv, b_qkv, W_o, b_o):
    import ml_dtypes
    from concourse.bass_utils import run_bass_kernel_spmd

    bf = ml_dtypes.bfloat16
    x = np.asarray(x, dtype=np.float32)
    W_qkv = np.asarray(W_qkv, dtype=np.float32)
    b_qkv = np.asarray(b_qkv, dtype=np.float32)
    W_o = np.asarray(W_o, dtype=np.float32)
    b_o = np.asarray(b_o, dtype=np.float32)

    # idm: [128, 0:128] identity; [128, 128:256] causal mask: -240 where the
    # [ktok_row, qtok_col] block entry violates j >= i (q < k).
    ident = np.eye(128, dtype=np.float32)
    m = np.where(
        np.arange(128)[None, :] >= np.arange(128)[:, None], 0.0, -240.0
    ).astype(np.float32)
    tri = np.triu(np.ones((128, 128), dtype=np.float32))
    idm = np.concatenate([ident, m, tri], axis=1)

    in_maps = []
    for c in range(N_CORES):
        b, g = divmod(c, 2)
        cs = slice(CL * g, CL * (g + 1))
        W_q_c = W_qkv[:, 0:C][:, cs]
        W_k_c = W_qkv[:, C : 2 * C][:, cs]
        W_v_c = W_qkv[:, 2 * C : 3 * C][:, cs]
        b_v_c = b_qkv[2 * C : 3 * C][cs]
        W_o_c = W_o[cs, :]
        # V-bias folds into the output bias: softmax rows sum to 1, so
        # P @ (1 b_v^T) = 1 b_v^T, and (O + 1 b_v^T) W_o = O W_o + 1 (b_v^T W_o).
        bo2 = 0.5 * b_o + b_v_c @ W_o_c
        in_maps.append(
            {
                "xt": np.ascontiguousarray(
                    x[b].T.reshape(KC, 128, N).transpose(1, 0, 2)
                ).astype(bf),
                "w_q": np.ascontiguousarray(
                    W_q_c.reshape(KC, 128, PAIRS, 128).transpose(1, 2, 0, 3)
                ).astype(bf),
                "w_k": np.ascontiguousarray(
                    W_k_c.reshape(KC, 128, PAIRS, 128).transpose(1, 2, 0, 3)
                ).astype(bf),
                "w_v": np.ascontiguousarray(
                    W_v_c.reshape(KC, 128, CL).transpose(1, 0, 2)
                ).astype(bf),
                "w_o": np.ascontiguousarray(
                    W_o_c.reshape(PAIRS, 128, C).transpose(1, 0, 2)
                ).astype(bf),
                "b_q": np.ascontiguousarray(
                    b_qkv[0:C][cs].reshape(PAIRS, 128).T
                ).astype(np.float32),
                "b_o2": np.ascontiguousarray(bo2[None, :]).astype(np.float32),
                "idm": np.ascontiguousarray(idm).astype(bf),
            }
        )

    nc = _get_nc()
    trace = bool(int(os.environ.get("BASS_KERNEL_TRACE", "0")))
    tmpdir = os.environ.get("BASS_KERNEL_TRACE_DIR") or None
    res = run_bass_kernel_spmd(
        nc, in_maps, list(range(N_CORES)), trace=trace, tmpdir=tmpdir
    )
    kernel.last_result = res

    full = np.empty((B, N, C), dtype=np.float32)
    chunks = [(0, 512), (512, 512), (1024, 512)]
    outs = [np.asarray(res.results[c]["out"], dtype=np.float32) for c in range(N_CORES)]
    for c in range(N_CORES):
        b, rank = divmod(c, 2)
        o = outs[c]
        out_r = 0
        for t0, tn in chunks:
            h = tn // 2
            full[b, t0 + rank * h : t0 + (rank + 1) * h, :] = o[out_r : out_r + h, :]
            out_r += h
    for b in range(B):
        full[b, 1536:2048, :] = outs[2 * b][768:1280, :] + outs[2 * b + 1][768:1280, :]
    return full


kernel.last_result = None
